# revision 15
# baseline (speedup 1.0000x reference)
"""VQ codebook argmin kernel for Trainium2 (8 NeuronCores, data-parallel on B).

Problem: x [32768, 512] f32, centroids [4096, 512] f32 ->
         argmin_k ||x_b - c_k||^2 = argmin_k (csq_k - 2 x.c_k)  -> [32768] int32

Sharding: x split along B into 8 shards of 4096 rows; centroids replicated.

Strategy (top8-pairs): ONE f32r (TF32-like, 1 cyc/col, ~2^-11 mantissa)
matmul pass computes nd = 2*x.c - csq approximately, with csq folded into the
GEMM as a 5th one-row matmul per k-chunk (lhsT = ones[1,128], rhs =
-csq[1,512]).  The Act engine drains PSUM to fp16 SBUF; the Pool engine
max-reduces adjacent centroid pairs (4096 -> 2048); DVE extracts the top-8
(pair-value, pair-index) per row via InstMax/InstMaxIndex.  The host then
exactly disambiguates the winning pair (2 f64 dot products per row), and for
rows whose top-1/top-2 pair gap is below TAU (a certified bound on
|device_nd - exact_nd|) rechecks all 16 candidate centroids; rows whose
top-1/top-8 spread is below TAU (none in practice) fall back to a full-row
exact argmin.  Correctness certificate: any centroid outside the top-8 pairs
sits in a pair with device value <= v8 <= v1 - TAU, so its exact value is
below the exact value of the top pair's best member — it cannot be the
argmin.

Engine split per 128-row b-tile: PE does 4 f32 transposes of x + 40 f32r
matmuls; Act copies transposed x and drains the 4 PSUM pair-bank groups;
Pool pair-reduces; DVE does max + max_index.  centroids are transposed once
on PE in the prologue and kept resident in SBUF as f32r(2c) ([128, 4, 4096],
64KB/partition).
"""
import sys

sys.path.insert(0, "/opt/trn_rl_repo")

import numpy as np

import concourse.bacc as bacc
import concourse.mybir as mybir
import concourse.tile as tile
from concourse.bass_utils import run_bass_kernel_spmd
from concourse.masks import make_identity

P = 128
D = 512
K = 4096
B = 32768
N_CORES = 8
B_SH = B // N_CORES          # 4096 rows per core
NBT = B_SH // P              # 32 b-tiles per core
DC = D // P                  # 4 contraction chunks
KC_SIZE = 512                # k-chunk (PSUM free dim)
NKC = K // KC_SIZE           # 8 k-chunks

F32 = mybir.dt.float32
F32R = mybir.dt.float32r
F16 = mybir.dt.float16
U16 = mybir.dt.uint16
AL = mybir.AluOpType
ACT = mybir.ActivationFunctionType

# Host-repair margin: |device_nd - exact_nd| <= eps.  Components: f32r GEMM
# error (~1.5e-2 measured on this data), fp16 output rounding (<=0.25 for
# |nd|<1024), f32r-rounded csq (~0.4 incl the f32r prologue matmul), f32
# accum noise.  TAU = 2*eps with safety.
TAU = 2.0


def build_bass_top8(b_sh: int = B_SH, k: int = K, repeat: int = 1):
    """repeat > 1 re-emits the full per-call body (c-load/transpose/csq +
    main loop + output DMA) that many times into one NEFF, rewriting the
    same persistent tiles — used by the benchmark to amortize the ~3.5ms
    axon dispatch overhead over R honest iterations."""
    B_SH, K = b_sh, k
    NBT = B_SH // P
    NKC = K // KC_SIZE

    nc = bacc.Bacc("TRN2", target_bir_lowering=False, debug=False)

    x_d = nc.dram_tensor("x_shard", [B_SH, D], F32, kind="ExternalInput")
    c_d = nc.dram_tensor("centroids", [K, D], F32, kind="ExternalInput")
    val_d = nc.dram_tensor("out_val8", [B_SH, 8], F16, kind="ExternalOutput")
    idx_d = nc.dram_tensor("out_idx8", [B_SH, 8], U16, kind="ExternalOutput")

    with tile.TileContext(nc) as tc:
        with (
            tc.tile_pool(name="persist", bufs=1) as persist,
            tc.tile_pool(name="cin", bufs=2) as cin,
            tc.tile_pool(name="xin", bufs=3) as xin,
            tc.tile_pool(name="xtp", bufs=3) as xtp,
            tc.tile_pool(name="nd", bufs=2) as ndp,
            tc.tile_pool(name="pm", bufs=2) as pmp,
            tc.tile_pool(name="scratch", bufs=2) as scratch,
            # 2 PSUM banks per mm tile x 2 bufs + 3 transpose banks = 7 of 8
            tc.tile_pool(name="mm_psum", bufs=2, space="PSUM") as mm_psum,
            tc.tile_pool(name="tr_psum", bufs=3, space="PSUM") as tr_psum,
        ):
            ident = persist.tile([P, P], F32)
            make_identity(nc, ident)
            ones = persist.tile([P, P], F32)
            nc.vector.memset(ones[:], 1.0)
            ones_r = persist.tile([P, P], F32R)
            nc.scalar.activation(ones_r[:], ones[:], ACT.Copy)

            cT = persist.tile([P, DC, K], F32R)
            negcsq = persist.tile([P, K], F32R)
            val_all = persist.tile([P, NBT, 8], F16)
            idx_all = persist.tile([P, NBT, 8], U16)
            for _ in range(repeat):
                _emit_body(nc, cin, xin, xtp, ndp, pmp, scratch, mm_psum,
                           tr_psum, ident, ones_r, cT, negcsq,
                           val_all, idx_all, x_d, c_d, val_d, idx_d,
                           B_SH, K, NBT, NKC)

    nc.compile()
    return nc


def _emit_body(nc, cin, xin, xtp, ndp, pmp, scratch, mm_psum, tr_psum,
               ident, ones_r, cT, negcsq, val_all, idx_all,
               x_d, c_d, val_d, idx_d, B_SH, K, NBT, NKC):
    # ---- transpose centroids: cT[dp, dc, k] = 2*c[k, dc*128+dp],
    # f32r-rounded by the Act copy (producers of f32r matmul inputs must
    # round)
    for t in range(K // P):
        raw = cin.tile([P, D], F32, tag="raw_c")
        nc.sync.dma_start(raw[:], c_d.ap()[t * P:(t + 1) * P, :])
        pst = tr_psum.tile([P, DC, P], F32, tag="tr")
        for dc in range(DC):
            nc.tensor.transpose(pst[:, dc, :], raw[:, dc * P:(dc + 1) * P],
                                ident[:])
        nc.scalar.activation(cT[:, :, t * P:(t + 1) * P], pst[:],
                             ACT.Copy, scale=2.0)

    # ---- negcsq[p, k] = -sum_d c[k, d]^2 (row 0 feeds the csq-fold matmul)
    for g in range(NKC // 2):
        ps = mm_psum.tile([P, 2, KC_SIZE], F32, tag="mm")
        for u in range(2):
            j = 2 * g + u
            ksl = slice(j * KC_SIZE, (j + 1) * KC_SIZE)
            sq = scratch.tile([P, DC, KC_SIZE], F32R, tag="sq")
            # cT holds 2c -> Square(0.5 * cT) = c^2
            nc.scalar.activation(sq[:], cT[:, :, ksl].bitcast(F32),
                                 ACT.Square, scale=0.5)
            for dc in range(DC):
                nc.tensor.matmul(
                    ps[:, u, :], lhsT=ones_r[:], rhs=sq[:, dc, :],
                    start=(dc == 0), stop=(dc == DC - 1),
                )
        nc.scalar.activation(
            negcsq[:, 2 * g * KC_SIZE:(2 * g + 2) * KC_SIZE],
            ps[:].rearrange("p u k -> p (u k)"), ACT.Copy, scale=-1.0)

    def x_load(i):
        rawx = xin.tile([P, D], F32, tag="raw_x")
        nc.sync.dma_start(rawx[:], x_d.ap()[i * P:(i + 1) * P, :])
        return rawx

    def x_transpose(rawx):
        pst = tr_psum.tile([P, DC, P], F32, tag="tr")
        for dc in range(DC):
            nc.tensor.transpose(pst[:, dc, :], rawx[:, dc * P:(dc + 1) * P],
                                ident[:])
        xT = xtp.tile([P, DC, P], F32R, tag="xT")
        nc.scalar.activation(xT[:], pst[:], ACT.Copy)
        return xT

    def tile_body(i, xT):
        nd = ndp.tile([P, K], F16, tag="nd")
        for g in range(NKC // 2):
            ps = mm_psum.tile([P, 2, KC_SIZE], F32, tag="mm")
            for u in range(2):
                j = 2 * g + u
                ksl = slice(j * KC_SIZE, (j + 1) * KC_SIZE)
                for dc in range(DC):
                    nc.tensor.matmul(
                        ps[:, u, :], lhsT=xT[:, dc, :], rhs=cT[:, dc, ksl],
                        start=(dc == 0), stop=False,
                    )
                # csq fold: ps += ones[1,128] . (-csq)[1,512]
                nc.tensor.matmul(
                    ps[:, u, :], lhsT=ones_r[0:1, :], rhs=negcsq[0:1, ksl],
                    start=False, stop=True,
                )
            nc.scalar.activation(
                nd[:, 2 * g * KC_SIZE:(2 * g + 2) * KC_SIZE],
                ps[:].rearrange("p u k -> p (u k)"), ACT.Copy)
        # pair-reduce 4096 -> 2048 (pair i = centroids {i, i+2048}; both
        # operands contiguous fp16 so the 2x DVE mode applies), then top-8
        # pairs via Max/MaxIndex
        pm = pmp.tile([P, K // 2], F16, tag="pm")
        nc.vector.tensor_tensor(out=pm[:], in0=nd[:, :K // 2],
                                in1=nd[:, K // 2:], op=AL.max)
        nc.vector.max(val_all[:, i, :], pm[:])
        nc.vector.max_index(idx_all[:, i, :], val_all[:, i, :], pm[:])

    # software-pipelined main loop: load i+2, transpose i+1, body i
    raws = {i: x_load(i) for i in range(min(2, NBT))}
    xTs = {0: x_transpose(raws.pop(0))} if NBT else {}
    for i in range(NBT):
        if i + 2 < NBT:
            raws[i + 2] = x_load(i + 2)
        if i + 1 < NBT:
            xTs[i + 1] = x_transpose(raws.pop(i + 1))
        tile_body(i, xTs.pop(i))

    nc.sync.dma_start(
        val_d.ap().rearrange("(t p) j -> p t j", p=P), val_all[:]
    )
    nc.sync.dma_start(
        idx_d.ap().rearrange("(t p) j -> p t j", p=P), idx_all[:]
    )


_NC = None


def _host_repair(x, centroids, csq, val8, idx8):
    """Exact-repair the device top-8 pair candidates.  val8 [n,8] f16
    descending pair-max nd values; idx8 [n,8] u16 pair indices (pair i =
    centroids {2i, 2i+1}).  Returns int32 argmin indices."""
    val = val8.astype(np.float64)
    pairs = idx8.astype(np.int64)
    n = val.shape[0]
    cd = centroids.astype(np.float64)

    # always disambiguate the winning pair exactly (2 dots per row);
    # pair i = centroids {i, i + K/2}
    k2 = np.stack([pairs[:, 0], pairs[:, 0] + K // 2], axis=1)      # [n, 2]
    nd2 = 2.0 * np.einsum("nd,njd->nj", x.astype(np.float64), cd[k2]) - csq[k2]
    ans = k2[np.arange(n), np.argmax(nd2, axis=1)]  # tie -> lower k (first)

    gap1 = val[:, 0] - val[:, 1]
    flagged = np.nonzero(gap1 <= TAU)[0]
    if flagged.size:
        full_rows = flagged[val[flagged, 0] - val[flagged, 7] <= TAU]
        kc = np.repeat(pairs[flagged], 2, axis=1)
        kc[:, 1::2] += K // 2                           # [nf, 16] candidates
        xf = x[flagged].astype(np.float64)
        nd_exact = 2.0 * np.einsum("nd,njd->nj", xf, cd[kc]) - csq[kc]
        order = np.lexsort((kc, -nd_exact), axis=1)[:, 0]
        ans[flagged] = kc[np.arange(kc.shape[0]), order]
        if full_rows.size:
            xr = x[full_rows].astype(np.float64)
            ndf = 2.0 * xr @ cd.T - csq[None, :]
            ans[full_rows] = np.argmax(
                ndf - 1e-12 * np.arange(ndf.shape[1]), axis=1
            )
    return ans.astype(np.int32)


def kernel(x: np.ndarray, centroids: np.ndarray) -> np.ndarray:
    global _NC
    if _NC is None:
        _NC = build_bass_top8()
    x = np.ascontiguousarray(x, dtype=np.float32)
    centroids = np.ascontiguousarray(centroids, dtype=np.float32)
    in_maps = [
        {"x_shard": x[c * B_SH:(c + 1) * B_SH], "centroids": centroids}
        for c in range(N_CORES)
    ]
    res = run_bass_kernel_spmd(_NC, in_maps, core_ids=list(range(N_CORES)))
    csq = np.sum(centroids.astype(np.float64) ** 2, axis=1)
    outs = []
    for c in range(N_CORES):
        outs.append(_host_repair(
            x[c * B_SH:(c + 1) * B_SH], centroids, csq,
            res.results[c]["out_val8"], res.results[c]["out_idx8"],
        ))
    return np.concatenate(outs)
